# revision 41
# baseline (speedup 1.0000x reference)
"""Multi-head attention (B=4, S=2048, D=1024, H=16) on 8 trn2 NeuronCores.

Sharding: 8 cores = 4 batches x 2 head-groups. Core c handles batch c//2 and
heads [8g, 8g+8) where g = c%2 (tensor-parallel: Wq/Wk/Wv column-sliced,
Wo row-sliced). Each core returns a partial output [S, D]; the host sums the
two head-group partials per batch.

Key-compaction: the mask drops a key entirely (exp(-1e9) == 0), so the host
gathers only the unmasked key rows of k/v per batch (padded to a 128-multiple
SK; pad slots are masked out on device). For the ~50% random mask this nearly
halves all K-side work (K/V projection, scores, exp, context).

All x / W tensors travel as bf16 (host casts): half the DMA traffic, PE
transposes run at 1 cycle/row instead of 2, and no on-device f32->bf16 casts.

Per-core dataflow (everything stays transposed until the output projection):
  xT tiles (PE transpose, bf16) -> Q.T/K.T = W.T @ X.T, V natural (ones
  column appended) -> scores.T = K @ Q.T -> exp+mask+scale in one ACT op
  -> ctxU.T = V'.T @ expS.T (last row = softmax denominator) -> normalize
  (fast-approx reciprocal) -> out = ctx.T.T @ Wo + bo.
"""

import sys

if "/opt/trn_rl_repo" not in sys.path:
    sys.path.append("/opt/trn_rl_repo")

import numpy as np
import ml_dtypes

import concourse.bass as bass
import concourse.bacc as bacc
import concourse.tile as tile
from concourse import mybir
from concourse.bass import ts

F32 = mybir.dt.float32
BF16 = mybir.dt.bfloat16
FP8 = mybir.dt.float8e4
I32 = mybir.dt.int32
EXP = mybir.ActivationFunctionType.Exp
DBLROW = mybir.MatmulPerfMode.DoubleRow

P = 128
NPBF = ml_dtypes.bfloat16
NPF8 = ml_dtypes.float8_e4m3fn


def build_nc(S=2048, D=1024, DL=512, HD=64, SK=1152):
    """Per-core Bass program. DL = local out dim (heads*HD), SK = key len."""
    ST = S // P  # q token tiles
    SKT = SK // P  # key token tiles
    KD = D // P  # contraction tiles over D
    MT = DL // P  # local d-col tiles
    HL = DL // HD  # local heads
    HPT = P // HD  # heads per 128-partition tile (2)
    NCH = min(512, S)  # projection token-chunk
    QS = min(1024, S)  # attention q superchunk (<=2 psum banks)
    QH = min(512, QS)  # one-bank half
    NH = QS // QH
    NQ = S // QS  # q-superchunks
    OC = min(512, D)  # out-proj col chunk
    scale = float(1.0 / (np.sqrt(np.float32(HD)) + 1e-8))

    # K-side token chunks (SK may not be a NCH multiple)
    kchunks = []
    o = 0
    while o < SK:
        w_ = min(NCH, SK - o)
        kchunks.append((o, w_))
        o += w_

    nc = bacc.Bacc("TRN2", target_bir_lowering=False, debug=False)

    xq = nc.dram_tensor("xq", [S, D], BF16, kind="ExternalInput")
    xk = nc.dram_tensor("xk", [SK, D], BF16, kind="ExternalInput")
    xv = nc.dram_tensor("xv", [SK, D], BF16, kind="ExternalInput")
    # packed per-partition consts: [bk (MT) | bq (MT) | mask-as-f32 (SKT)]
    cst = nc.dram_tensor("cst", [P, 2 * MT + SKT], F32, kind="ExternalInput")
    wq = nc.dram_tensor("wq", [D, DL], BF16, kind="ExternalInput")
    wk = nc.dram_tensor("wk", [D, DL], BF16, kind="ExternalInput")
    wv = nc.dram_tensor("wv", [D, DL], BF16, kind="ExternalInput")
    wo = nc.dram_tensor("wo", [DL, D], BF16, kind="ExternalInput")
    bv = nc.dram_tensor("bv", [1, DL], F32, kind="ExternalInput")
    bo = nc.dram_tensor("bo", [1, D], F32, kind="ExternalInput")
    out = nc.dram_tensor("out", [S, D], F32, kind="ExternalOutput")

    with tile.TileContext(nc) as tc, nc.allow_low_precision("bf16 operands are rounded by design"):
        with (
            tc.tile_pool(name="pers", bufs=1) as pers,
            tc.tile_pool(name="wpool", bufs=3) as wpool,
            tc.tile_pool(name="exp", bufs=6) as ex_pool,
            tc.tile_pool(name="osb", bufs=3) as osb_pool,
            tc.tile_pool(name="small", bufs=2) as small,
        ):
            # ---- constants. Weight/bias loads go through the Scalar
            # engine's DMA queue; the small per-partition consts (bk, bq,
            # mask) come in one packed DMA. ----
            csts = pers.tile([P, 2 * MT + SKT], F32, tag="csts")
            bks = csts[:, 0:MT]
            bqs = csts[:, MT : 2 * MT]
            mb = pers.tile([P, SKT], F32, tag="mb")

            def load_consts():
                nc.scalar.dma_start(csts[:], cst[:, :])
                nc.vector.tensor_scalar_mul(
                    mb[:], csts[:, 2 * MT : 2 * MT + SKT], -1.0e9
                )

            bvs = pers.tile([1, DL], F32, tag="bvs")
            bos = pers.tile([1, D], F32, tag="bos")
            bvb = pers.tile([P, DL], F32, tag="bvb")
            bob = pers.tile([P, D], F32, tag="bob")

            def late_consts():
                nc.scalar.dma_start(bvs[:], bv[:, :])
                nc.scalar.dma_start(bos[:], bo[:, :])
                nc.gpsimd.partition_broadcast(bvb[:], bvs[0:1, :])
                nc.gpsimd.partition_broadcast(bob[:], bos[0:1, :])

            # persistent activation stores
            KT = [pers.tile([P, SK], BF16, tag=f"kt{m}", name=f"kt{m}") for m in range(MT)]
            QT = [pers.tile([P, S], BF16, tag=f"qt{m}", name=f"qt{m}") for m in range(MT)]
            CT = [pers.tile([P, S], BF16, tag=f"ct{m}", name=f"ct{m}") for m in range(MT)]
            VP = [pers.tile([P, HL * (HD + 1)], BF16, tag=f"vp{t}", name=f"vp{t}") for t in range(SKT)]

            # x.T via the DMA-engine transpose (XT[p, kk, t] = x[t, kk*128+p]):
            # no PE transposes, no PSUM staging. DMA-transposes barrier
            # against other DMAs, so don't interleave them.
            XK = pers.tile([P, KD, SK], BF16, tag="XK")
            XV = pers.tile([P, KD, SK], BF16, tag="XV")
            XQ = pers.tile([P, KD, S], BF16, tag="XQ")

            def load_w(wdram, dt):
                w = wpool.tile([P, KD, DL], dt, tag="w", name="w")
                nc.scalar.dma_start(w[:], wdram.rearrange("(k p) n -> p k n", p=P))
                return w

            wos = pers.tile([P, MT, D], BF16, tag="wos")

            def proj_mm_units(xt, wsb, bias_sb, dst_tiles, tok0, ntok, acc_pool):
                """dst[m][:, tok0:tok0+ntok] = (x @ w + b).T; yields per m."""
                for m in range(MT):
                    acc = acc_pool.tile([P, NCH], F32, tag="acc")
                    for kk in range(KD):
                        nc.tensor.matmul(
                            acc[:, 0:ntok],
                            lhsT=wsb[:, kk, ts(m, P)],
                            rhs=xt[:, kk, tok0 : tok0 + ntok],
                            start=(kk == 0),
                            stop=(kk == KD - 1),
                        )
                    nc.vector.tensor_scalar_add(
                        dst_tiles[m][:, tok0 : tok0 + ntok], acc[:, 0:ntok], bias_sb[:, m : m + 1]
                    )
                    yield

            def vproj(wsb, acc_pool):
                """VP[t][:, h*(HD+1):+HD] = (xv @ wv + bv)[t-tile, h-slice]."""
                for t in range(SKT):
                    acc = acc_pool.tile([P, DL], F32, tag="acc")
                    for kk in range(KD):
                        nc.tensor.matmul(
                            acc[:],
                            lhsT=XV[:, kk, ts(t, P)],
                            rhs=wsb[:, kk, :],
                            start=(kk == 0),
                            stop=(kk == KD - 1),
                        )
                    for h in range(HL):
                        nc.vector.tensor_add(
                            VP[t][:, h * (HD + 1) : h * (HD + 1) + HD],
                            acc[:, ts(h, HD)],
                            bvb[:, ts(h, HD)],
                        )

            def attention(qq, sc_pool, cx_pool, filler=None, pump_every=8):
                it = 0
                pending = []  # deferred normalize work (recip/broadcast/mul)
                for hp in range(HL // HPT):  # head pairs share a KT/QT tile
                    for q5 in range(NH):
                        col0 = qq * QS + q5 * QH
                        cxs = [
                            cx_pool.tile([HD + 1, QH], F32, tag="cx", name="cx")
                            for _ in range(HPT)
                        ]
                        for kt in range(SKT):
                            # one PSUM supertile holds both heads' score chunk;
                            # the two K=64 matmuls run concurrently (row groups
                            # 0-63 / 64-127), one ACT exp covers both
                            sc = sc_pool.tile([P, HPT * QH], F32, tag="sc")
                            for u in range(HPT):
                                mo = u * HD
                                nc.tensor.matmul(
                                    sc[:, ts(u, QH)],
                                    lhsT=KT[hp][mo : mo + HD, ts(kt, P)],
                                    rhs=QT[hp][mo : mo + HD, col0 : col0 + QH],
                                    start=True,
                                    stop=True,
                                )
                            ex = ex_pool.tile([P, HPT * QH], BF16, tag="ex")
                            nc.scalar.activation(
                                ex[:], sc[:], EXP, bias=mb[:, kt : kt + 1], scale=scale
                            )
                            for u in range(HPT):
                                h = hp * HPT + u
                                nc.tensor.matmul(
                                    cxs[u][:],
                                    lhsT=VP[kt][:, h * (HD + 1) : (h + 1) * (HD + 1)],
                                    rhs=ex[:, ts(u, QH)],
                                    start=(kt == 0),
                                    stop=(kt == SKT - 1),
                                )
                            it += 1
                            if filler is not None and it % pump_every == 0:
                                next(filler, None)
                        prev_tails = pending
                        pending = []
                        for u in range(HPT):
                            mo = u * HD
                            # the cheap DVE copy (emitted now, ahead of the
                            # previous unit's reciprocals in DVE order) frees
                            # the ctx PSUM slot; recip/broadcast/mul are
                            # deferred one unit so nothing waits on them
                            stg = small.tile([HD + 1, QH], F32, tag="stg", name="stg", bufs=4)
                            nc.vector.tensor_copy(out=stg[:], in_=cxs[u][:])

                            def tail(hp=hp, mo=mo, col0=col0, stg=stg):
                                # denominator row lives at partition HD; the
                                # custom gpsimd/DVE ops read absolute partition
                                # 0 on HW, so DMA it down to a base-0 tile first
                                rec = small.tile([1, QH], F32, tag="rec", name="rec", bufs=2)
                                nc.sync.dma_start(rec[0:1, :], stg[HD : HD + 1, :])
                                dnb = small.tile([HD, QH], F32, tag="dnb", name="dnb", bufs=2)
                                nc.gpsimd.partition_broadcast(dnb[:], rec[0:1, :])
                                bcs = small.tile([HD, QH], F32, tag="bcs", bufs=2)
                                nc.vector.reciprocal_approx_fast(bcs[:], dnb[:])
                                if mo == 0:
                                    nc.vector.tensor_mul(
                                        CT[hp][0:HD, col0 : col0 + QH],
                                        stg[0:HD, :],
                                        bcs[:],
                                    )
                                else:
                                    tmp = small.tile([HD, QH], BF16, tag="tmp")
                                    nc.vector.tensor_mul(tmp[:], stg[0:HD, :], bcs[:])
                                    nc.sync.dma_start(
                                        CT[hp][mo : mo + HD, col0 : col0 + QH], tmp[:]
                                    )

                            pending.append(tail)
                        for fn in prev_tails:
                            fn()

                for fn in pending:
                    fn()

            def outproj_units(qq, acc_pool):
                t0 = qq * (QS // P)
                for t in range(t0, t0 + QS // P):
                    for c in range(D // OC):
                        po = acc_pool.tile([P, OC], F32, tag="acc", name="po")
                        for dd in range(MT):
                            nc.tensor.matmul(
                                po[:],
                                lhsT=CT[dd][:, ts(t, P)],
                                rhs=wos[:, dd, ts(c, OC)],
                                start=(dd == 0),
                                stop=(dd == MT - 1),
                            )
                        osb = osb_pool.tile([P, OC], F32, tag="osb")
                        nc.vector.tensor_add(osb[:], po[:], bob[:, ts(c, OC)])
                        nc.sync.dma_start(out[ts(t, P), ts(c, OC)], osb[:])
                        yield

            CPQ = QS // NCH  # projection chunks per q-superchunk

            # ---- phase 1: K.T and V' (full-S prerequisites of attention) ----
            # DMA-transposes barrier against every other DMA (each waits all
            # prior completions and blocks later ones), so the whole DMA
            # program is one chain ordered by each transfer's deadline.
            with tc.tile_pool(name="ps1acc", bufs=6, space="PSUM") as ps1acc:
                for t in range(SKT):
                    nc.gpsimd.memset(VP[t][:], 1.0)
                ksplit = kchunks[0][1]
                nc.sync.dma_start_transpose(XK[:, :, 0:ksplit], xk[0:ksplit, :])
                wks = load_w(wk, BF16)
                load_consts()
                if ksplit < SK:
                    nc.sync.dma_start_transpose(
                        XK[:, :, ksplit:SK], xk[ksplit:SK, :]
                    )
                vsplit = min(512, SK)
                nc.sync.dma_start_transpose(XV[:, :, 0:vsplit], xv[0:vsplit, :])
                wvs = load_w(wv, BF16)
                if vsplit < SK:
                    nc.sync.dma_start_transpose(
                        XV[:, :, vsplit:SK], xv[vsplit:SK, :]
                    )
                late_consts()
                nc.sync.dma_start_transpose(XQ[:, :, 0:QS], xq[0:QS, :])
                wqs = load_w(wq, BF16)
                nc.scalar.dma_start(wos[:], wo.rearrange("(m p) n -> p m n", p=P))
                for qq in range(1, NQ):
                    nc.sync.dma_start_transpose(
                        XQ[:, :, qq * QS : (qq + 1) * QS],
                        xq[qq * QS : (qq + 1) * QS, :],
                    )
                for tok0, ntok in kchunks:
                    for _ in proj_mm_units(XK, wks, bks, KT, tok0, ntok, ps1acc):
                        pass
                vproj(wvs, ps1acc)

            # ---- phase 2: Q.T chunks, attention, out-proj ----
            with (
                tc.tile_pool(name="ps2acc", bufs=2, space="PSUM") as ps2acc,
                tc.tile_pool(name="ps2sc", bufs=2, space="PSUM") as ps2sc,
                tc.tile_pool(name="ps2cx", bufs=2, space="PSUM") as ps2cx,
            ):
                from itertools import chain

                n_att_its = (HL // HPT) * NH * SKT
                for nch in range(CPQ):
                    for _ in proj_mm_units(
                        XQ, wqs, bqs, QT, nch * NCH, NCH, ps2acc
                    ):
                        pass
                for qq in range(NQ):
                    if qq + 1 < NQ:
                        filler = chain.from_iterable(
                            proj_mm_units(XQ, wqs, bqs, QT, nch * NCH, NCH, ps2acc)
                            for nch in range((qq + 1) * CPQ, (qq + 2) * CPQ)
                        )
                        n_units = CPQ * MT
                    elif qq >= 1:
                        filler = outproj_units(qq - 1, ps2acc)
                        n_units = (QS // P) * (D // OC)
                    else:
                        filler = None
                        n_units = 1
                    attention(
                        qq,
                        ps2sc,
                        ps2cx,
                        filler,
                        pump_every=max(1, n_att_its // max(n_units, 1)),
                    )
                    if filler is not None:
                        for _ in filler:
                            pass
                if NQ >= 2:
                    for qq in range(NQ - 2):
                        for _ in outproj_units(qq, ps2acc):
                            pass

            # ---- phase 3: final out-proj; psum depth 2 is enough for the
            # epilogue (vector add + store) to trail by less than a unit ----
            with tc.tile_pool(name="ps3", bufs=2, space="PSUM") as ps3:
                for _ in outproj_units(NQ - 1, ps3):
                    pass

    nc.compile()
    return nc


_NC_CACHE = {}


def _get_nc(S, D, DL, HD, SK):
    key = (S, D, DL, HD, SK)
    if key not in _NC_CACHE:
        _NC_CACHE[key] = build_nc(S, D, DL, HD, SK)
    return _NC_CACHE[key]


def _shard_inputs(q, k, v, mask, Wq, bq, Wk, bk, Wv, bv, Wo, bo):
    q, k, v = np.asarray(q), np.asarray(k), np.asarray(v)
    mask = np.asarray(mask)
    Wq, Wk, Wv, Wo = np.asarray(Wq), np.asarray(Wk), np.asarray(Wv), np.asarray(Wo)
    bq, bk, bv, bo = np.asarray(bq), np.asarray(bk), np.asarray(bv), np.asarray(bo)

    B, S, D = q.shape  # 4, 2048, 1024
    G = 2  # head-groups (tensor-parallel factor); B*G = 8 cores
    DL = D // G
    MT = DL // P

    # key compaction: gather unmasked key rows, pad to a 128-multiple
    keep = [np.flatnonzero(mask[b, 0, 0] == 0) for b in range(B)]
    maxc = max(max((len(ix) for ix in keep), default=1), 1)
    SK = min(S, ((maxc + P - 1) // P) * P)
    SKT = SK // P

    f32 = np.float32
    xk_c, xv_c, msk_c = [], [], []
    for b in range(B):
        if SK == S:
            # fallback: no compaction, original order + original mask
            xk_c.append(np.ascontiguousarray(k[b], dtype=NPBF))
            xv_c.append(np.ascontiguousarray(v[b], dtype=NPBF))
            msk_c.append(np.ascontiguousarray(mask[b, 0, 0].reshape(SKT, P).T, dtype=f32))
        else:
            ix = keep[b][:SK]
            n = len(ix)
            kb = np.zeros((SK, D), dtype=NPBF)
            vb = np.zeros((SK, D), dtype=NPBF)
            kb[:n] = k[b][ix].astype(NPBF)
            vb[:n] = v[b][ix].astype(NPBF)
            mb_ = np.zeros((SK,), dtype=f32)
            mb_[n:] = 1
            xk_c.append(kb)
            xv_c.append(vb)
            msk_c.append(np.ascontiguousarray(mb_.reshape(SKT, P).T, dtype=f32))

    xq_b = [np.ascontiguousarray(q[b], dtype=NPBF) for b in range(B)]

    in_maps = []
    for c in range(B * G):
        b, g = c // G, c % G
        sl = slice(g * DL, (g + 1) * DL)
        bo_core = bo if g == 0 else np.zeros_like(bo)
        cst = np.concatenate(
            [
                bk[sl].reshape(MT, P).T.astype(f32),
                bq[sl].reshape(MT, P).T.astype(f32),
                msk_c[b],
            ],
            axis=1,
        )
        in_maps.append(
            {
                "xq": xq_b[b],
                "xk": xk_c[b],
                "xv": xv_c[b],
                "cst": np.ascontiguousarray(cst, dtype=f32),
                "wq": np.ascontiguousarray(Wq[:, sl].astype(NPBF)),
                "wk": np.ascontiguousarray(Wk[:, sl].astype(NPBF)),
                "wv": np.ascontiguousarray(Wv[:, sl].astype(NPBF)),
                "wo": np.ascontiguousarray(Wo[sl, :].astype(NPBF)),
                "bv": np.ascontiguousarray(bv[sl].reshape(1, DL), dtype=f32),
                "bo": np.ascontiguousarray(bo_core.reshape(1, D), dtype=f32),
            }
        )
    return in_maps, SK


def kernel(q, k, v, mask, Wq, bq, Wk, bk, Wv, bv, Wo, bo):
    from concourse.bass_utils import run_bass_kernel_spmd

    q = np.asarray(q)
    B, S, D = q.shape  # 4, 2048, 1024
    G = 2
    in_maps, SK = _shard_inputs(q, k, v, mask, Wq, bq, Wk, bk, Wv, bv, Wo, bo)
    nc = _get_nc(S, D, D // G, 64, SK)

    res = run_bass_kernel_spmd(nc, in_maps, core_ids=list(range(B * G)))
    parts = [r["out"] for r in res.results]
    outf = np.stack([parts[b * G] + parts[b * G + 1] for b in range(B)], axis=0)
    return outf.astype(np.float32)
